# revision 11
# baseline (speedup 1.0000x reference)
"""Trainium2 Bass kernel for ComputeAngleInput (GNN angular descriptor).

Math (per center c with n=16 neighbors, F=32 features):
  d_jk[j,k]  = |xyz_j[j] - xyz_j[k]|
  tij[j,k]   = dist[c,j];  tik[j,k] = dist[c,k]
  tjk[j,k]   = (d_jk - max(tij,tik) + min(tij,tik)) / (2*min(tij,tik))
  row(j,k)   = [tij, tik, tjk, emb_i(32), emb_j[j]/tij (32), emb_j[k]/tik (32)]
  output     = rows for all j != k (240 rows of 99 floats), plus centers=atom_i_idx

Strategy: data-parallel over centers across 8 cores (1250 each = 9 full
128-partition tiles + one 98-partition tile).  Host does the tiny index
gathers (atoms_xyz / embed_table lookups, ~24MB) and packs one 608-float
row per center; the device kernel does the O(C*n*n*(3+3F)) = 950MB
materialization, which is the memory-roofline term.  One center per
SBUF partition; the [256,99] feature block is built with broadcast
(stride-0) copies split across DVE/ACT, and the off-diagonal 240 rows
go to HBM as two big strided DMAs per tile (flat rows [0,136) and
[136,256); diagonals sit every 17th flat row, so "skip every 17th row"
is one regular access pattern per chunk).  The two output streams ride
different DMA paths (SP HWDGE ring / GpSimd SWDGE) so transfers
overlap; input loads ride the ACT HWDGE ring.
"""

import numpy as np

import concourse.bacc as bacc
import concourse.mybir as mybir
from concourse import tile
from concourse.bass_utils import run_bass_kernel_spmd

F32 = mybir.dt.float32

C_TOT = 10000
N = 16
F = 32
NCORES = 8
P = 128
C_CORE = C_TOT // NCORES             # 1250 centers per core
ROW_IN = 16 + 3 * 16 + 32 + N * F    # 608 floats per packed input row
NROW = N * (N - 1)                   # 240 output rows per center
NF = 3 + 3 * F                       # 99
ROW_OUT = NROW * NF                  # 23760 floats per center

_CACHED_NC = None


def _emit_tile(nc, io_pool, tmp_pool, ang_pool, inp, out, c0, p):
    """Emit one tile: centers [c0, c0+p), one center per partition."""
    TT = mybir.AluOpType
    ACTF = mybir.ActivationFunctionType

    pk = io_pool.tile([P, ROW_IN], F32, tag="pk")
    nc.sync.dma_start(out=pk[:p, :], in_=inp[c0 : c0 + p, :])

    dist = pk[:p, 0:16]
    xs = pk[:p, 16:32]
    ys = pk[:p, 32:48]
    zs = pk[:p, 48:64]
    embi = pk[:p, 64:96]
    embj = pk[:p, 96:608]

    # 1 / dist  (dist in [0.5, 4.5], no zero risk)
    invd = tmp_pool.tile([P, 16], F32, tag="invd")
    nc.vector.reciprocal(invd[:p, :], dist)

    # emb_j[j,f] / dist[j]
    embjs = tmp_pool.tile([P, 512], F32, tag="embjs")
    nc.vector.tensor_tensor(
        out=embjs[:p, :].rearrange("p (j f) -> p j f", j=N),
        in0=embj.rearrange("p (j f) -> p j f", j=N),
        in1=invd[:p, :].unsqueeze(2).broadcast_to([p, N, F]),
        op=TT.mult,
    )

    # pairwise squared distance among the 16 neighbors
    a = tmp_pool.tile([P, 256], F32, tag="ta")
    b = tmp_pool.tile([P, 256], F32, tag="tb")
    dsq = tmp_pool.tile([P, 256], F32, tag="dsq")
    av = a[:p, :].rearrange("p (j k) -> p j k", j=N)
    bv = b[:p, :].rearrange("p (j k) -> p j k", j=N)
    dsqv = dsq[:p, :].rearrange("p (j k) -> p j k", j=N)
    for i, w in enumerate((xs, ys, zs)):
        wj = w.unsqueeze(2).broadcast_to([p, N, N])
        wk = w.unsqueeze(1).broadcast_to([p, N, N])
        nc.vector.tensor_tensor(out=av, in0=wj, in1=wk, op=TT.subtract)
        if i == 0:
            nc.vector.tensor_tensor(out=dsqv, in0=av, in1=av, op=TT.mult)
        else:
            nc.vector.tensor_tensor(out=bv, in0=av, in1=av, op=TT.mult)
            nc.vector.tensor_tensor(out=dsqv, in0=dsqv, in1=bv, op=TT.add)

    djk = tmp_pool.tile([P, 256], F32, tag="djk")
    nc.scalar.sqrt(djk[:p, :], dsq[:p, :])

    dij_b = dist.unsqueeze(2).broadcast_to([p, N, N])
    dik_b = dist.unsqueeze(1).broadcast_to([p, N, N])
    maxd = tmp_pool.tile([P, 256], F32, tag="maxd")
    mind = tmp_pool.tile([P, 256], F32, tag="mind")
    nc.vector.tensor_tensor(
        out=maxd[:p, :].rearrange("p (j k) -> p j k", j=N),
        in0=dij_b, in1=dik_b, op=TT.max,
    )
    nc.vector.tensor_tensor(
        out=mind[:p, :].rearrange("p (j k) -> p j k", j=N),
        in0=dij_b, in1=dik_b, op=TT.min,
    )
    rmin = tmp_pool.tile([P, 256], F32, tag="rmin")
    nc.vector.reciprocal(rmin[:p, :], mind[:p, :])
    # (djk - maxd) * rmin * 0.5 + 0.5  ==  (djk - maxd + mind)/(2*mind)
    nc.vector.tensor_tensor(out=b[:p, :], in0=djk[:p, :], in1=maxd[:p, :], op=TT.subtract)
    nc.vector.tensor_tensor(out=a[:p, :], in0=b[:p, :], in1=rmin[:p, :], op=TT.mult)
    tjk = tmp_pool.tile([P, 256], F32, tag="tjk")
    nc.scalar.activation(tjk[:p, :], a[:p, :], ACTF.Copy, bias=0.5, scale=0.5)

    embjs_v = embjs[:p, :].rearrange("p (j f) -> p j f", j=N)

    # ---- chunk A: flat rows [0, 136) = (j 0..7, k all) + (j=8, k 0..7)
    # 137 rows: one pad row so the 8x(17->16) off-diag view stays in bounds
    angA = ang_pool.tile([P, 137 * NF], F32, tag="ang")
    a1 = angA[:p, 0 : 128 * NF].rearrange("p (j k c) -> p j k c", j=8, c=NF)
    a2 = angA[:p, 128 * NF : 136 * NF].rearrange("p (k c) -> p k c", c=NF)
    aflat = angA[:p, 0 : 136 * NF].rearrange("p (r c) -> p r c", c=NF)

    # col 0: tij = dist[j], col 1: tik = dist[k], col 2: tjk
    nc.vector.tensor_copy(
        out=a1[:, :, :, 0], in_=dist[:, 0:8].unsqueeze(2).broadcast_to([p, 8, N])
    )
    nc.vector.tensor_copy(out=a2[:, :, 0], in_=dist[:, 8:9].broadcast_to([p, 8]))
    nc.vector.tensor_copy(
        out=a1[:, :, :, 1], in_=dist.unsqueeze(1).broadcast_to([p, 8, N])
    )
    nc.vector.tensor_copy(out=a2[:, :, 1], in_=dist[:, 0:8])
    nc.vector.tensor_copy(
        out=a1[:, :, :, 2], in_=tjk[:p, 0:128].rearrange("p (j k) -> p j k", j=8)
    )
    nc.vector.tensor_copy(out=a2[:, :, 2], in_=tjk[:p, 128:136])
    nc.scalar.copy(
        out=aflat[:, :, 3 : 3 + F], in_=embi.unsqueeze(1).broadcast_to([p, 136, F])
    )
    nc.vector.tensor_copy(
        out=a1[:, :, :, 3 + F : 3 + 2 * F],
        in_=embjs_v[:, 0:8, :].unsqueeze(2).broadcast_to([p, 8, N, F]),
    )
    nc.vector.tensor_copy(
        out=a2[:, :, 3 + F : 3 + 2 * F],
        in_=embjs_v[:, 8:9, :].broadcast_to([p, 8, F]),
    )
    nc.scalar.copy(
        out=a1[:, :, :, 3 + 2 * F : 3 + 3 * F],
        in_=embjs_v.unsqueeze(1).broadcast_to([p, 8, N, F]),
    )
    nc.scalar.copy(out=a2[:, :, 3 + 2 * F : 3 + 3 * F], in_=embjs_v[:, 0:8, :])
    # off-diag rows 1..135 skipping every 17th -> output rows 0..127
    nc.sync.dma_start(
        out=out[c0 : c0 + p, 0 : 128 * NF].rearrange("c (g x) -> c g x", g=8),
        in_=angA[:p, NF : NF + 8 * 17 * NF].rearrange("p (g x) -> p g x", g=8)[
            :, :, 0 : 16 * NF
        ],
    )

    # ---- chunk B: flat rows [136, 256) = (j=8, k 8..15) + (j 9..15, k all)
    angB = ang_pool.tile([P, 120 * NF], F32, tag="ang")
    b1 = angB[:p, 0 : 8 * NF].rearrange("p (k c) -> p k c", c=NF)
    b2 = angB[:p, 8 * NF : 120 * NF].rearrange("p (j k c) -> p j k c", j=7, c=NF)
    bflat = angB[:p, :].rearrange("p (r c) -> p r c", c=NF)  # r=120

    nc.vector.tensor_copy(out=b1[:, :, 0], in_=dist[:, 8:9].broadcast_to([p, 8]))
    nc.vector.tensor_copy(
        out=b2[:, :, :, 0], in_=dist[:, 9:16].unsqueeze(2).broadcast_to([p, 7, N])
    )
    nc.vector.tensor_copy(out=b1[:, :, 1], in_=dist[:, 8:16])
    nc.vector.tensor_copy(
        out=b2[:, :, :, 1], in_=dist.unsqueeze(1).broadcast_to([p, 7, N])
    )
    nc.vector.tensor_copy(out=b1[:, :, 2], in_=tjk[:p, 136:144])
    nc.vector.tensor_copy(
        out=b2[:, :, :, 2], in_=tjk[:p, 144:256].rearrange("p (j k) -> p j k", j=7)
    )
    nc.scalar.copy(
        out=bflat[:, :, 3 : 3 + F], in_=embi.unsqueeze(1).broadcast_to([p, 120, F])
    )
    nc.vector.tensor_copy(
        out=b1[:, :, 3 + F : 3 + 2 * F],
        in_=embjs_v[:, 8:9, :].broadcast_to([p, 8, F]),
    )
    nc.vector.tensor_copy(
        out=b2[:, :, :, 3 + F : 3 + 2 * F],
        in_=embjs_v[:, 9:16, :].unsqueeze(2).broadcast_to([p, 7, N, F]),
    )
    nc.scalar.copy(out=b1[:, :, 3 + 2 * F : 3 + 3 * F], in_=embjs_v[:, 8:16, :])
    nc.scalar.copy(
        out=b2[:, :, :, 3 + 2 * F : 3 + 3 * F],
        in_=embjs_v.unsqueeze(1).broadcast_to([p, 7, N, F]),
    )
    # off-diag rows 1..119 skipping every 17th -> output rows 128..239
    nc.gpsimd.dma_start(
        out=out[c0 : c0 + p, 128 * NF : 240 * NF].rearrange("c (g x) -> c g x", g=7),
        in_=angB[:p, NF : NF + 7 * 17 * NF].rearrange("p (g x) -> p g x", g=7)[
            :, :, 0 : 16 * NF
        ],
    )


def _build_nc():
    nc = bacc.Bacc("TRN2", target_bir_lowering=False, debug=False)
    inp = nc.declare_dram_parameter("packed", [C_CORE, ROW_IN], F32, isOutput=False)
    out = nc.declare_dram_parameter("out", [C_CORE, ROW_OUT], F32, isOutput=True)

    with tile.TileContext(nc) as tc:
        with (
            tc.tile_pool(name="io", bufs=3) as io_pool,
            tc.tile_pool(name="tmp", bufs=3) as tmp_pool,
            tc.tile_pool(name="ang", bufs=3) as ang_pool,
        ):
            c0 = 0
            while c0 < C_CORE:
                p = min(P, C_CORE - c0)
                _emit_tile(nc, io_pool, tmp_pool, ang_pool, inp, out, c0, p)
                c0 += p
    nc.compile()
    return nc


def _get_nc():
    global _CACHED_NC
    if _CACHED_NC is None:
        _CACHED_NC = _build_nc()
    return _CACHED_NC


def _pack_inputs(atoms_xyz, embed_table, dist_ij, atom_types, atom_i_idx, atom_j_idx):
    atoms_xyz = np.asarray(atoms_xyz, dtype=np.float32)
    embed_table = np.asarray(embed_table, dtype=np.float32)
    dist_ij = np.asarray(dist_ij, dtype=np.float32)
    atom_types = np.asarray(atom_types)
    atom_i_idx = np.asarray(atom_i_idx)
    atom_j_idx = np.asarray(atom_j_idx)

    C = dist_ij.shape[0]
    packed = np.empty((C, ROW_IN), dtype=np.float32)
    emb = embed_table[atom_types]                     # [nAtoms, F]
    xyz_j = atoms_xyz[atom_j_idx]                     # [C, N, 3]
    packed[:, 0:16] = dist_ij
    packed[:, 16:64] = np.ascontiguousarray(
        xyz_j.transpose(0, 2, 1)
    ).reshape(C, 48)                                  # x(16) y(16) z(16)
    packed[:, 64:96] = emb[atom_i_idx]                # emb_i
    packed[:, 96:608] = emb[atom_j_idx].reshape(C, N * F)
    return packed


def kernel(atoms_xyz, embed_table, dist_ij, atom_types, atom_i_idx, atom_j_idx):
    packed = _pack_inputs(
        atoms_xyz, embed_table, dist_ij, atom_types, atom_i_idx, atom_j_idx
    )
    nc = _get_nc()
    in_maps = [
        {"packed": packed[i * C_CORE : (i + 1) * C_CORE]} for i in range(NCORES)
    ]
    res = run_bass_kernel_spmd(nc, in_maps, list(range(NCORES)))
    outs = [res.results[i]["out"] for i in range(NCORES)]
    full = np.concatenate(outs, axis=0).reshape(C_TOT, NROW, NF)
    centers = np.asarray(atom_i_idx).reshape(-1)
    return full, centers


# revision 13
# speedup vs baseline: 1.0887x; 1.0887x over previous
"""Trainium2 Bass kernel for ComputeAngleInput (GNN angular descriptor).

Math (per center c with n=16 neighbors, F=32 features):
  d_jk[j,k]  = |xyz_j[j] - xyz_j[k]|
  tij[j,k]   = dist[c,j];  tik[j,k] = dist[c,k]
  tjk[j,k]   = (d_jk - max(tij,tik) + min(tij,tik)) / (2*min(tij,tik))
  row(j,k)   = [tij, tik, tjk, emb_i(32), emb_j[j]/tij (32), emb_j[k]/tik (32)]
  output     = rows for all j != k (240 rows of 99 floats), plus centers=atom_i_idx

Strategy: data-parallel over centers across 8 cores (1250 each = 9 full
128-partition tiles + one 98-partition tile).  Host does the tiny index
gathers (atoms_xyz / embed_table lookups, ~24MB) and packs one 608-float
row per center; the device kernel does the O(C*n*n*(3+3F)) = 950MB
materialization, which is the memory-roofline term.  One center per
SBUF partition; the [256,99] feature block is built with broadcast
(stride-0) copies split across DVE/ACT, and the off-diagonal 240 rows
go to HBM as two big strided DMAs per tile (flat rows [0,136) and
[136,256); diagonals sit every 17th flat row, so "skip every 17th row"
is one regular access pattern per chunk).  The two output streams ride
different DMA paths (SP HWDGE ring / GpSimd SWDGE) so transfers
overlap; input loads ride the ACT HWDGE ring.
"""

import numpy as np

import concourse.bacc as bacc
import concourse.mybir as mybir
from concourse import tile
from concourse.bass_utils import run_bass_kernel_spmd

F32 = mybir.dt.float32

C_TOT = 10000
N = 16
F = 32
NCORES = 8
P = 128
C_CORE = C_TOT // NCORES             # 1250 centers per core
ROW_IN = 16 + 3 * 16 + 32 + N * F    # 608 floats per packed input row
NROW = N * (N - 1)                   # 240 output rows per center
NF = 3 + 3 * F                       # 99
ROW_OUT = NROW * NF                  # 23760 floats per center

_CACHED_NC = None


def _emit_tile(nc, io_pool, tmp_pool, ang_pool, inp, out, c0, p):
    """Emit one tile: centers [c0, c0+p), one center per partition."""
    TT = mybir.AluOpType
    ACTF = mybir.ActivationFunctionType

    pk = io_pool.tile([P, ROW_IN], F32, tag="pk")
    nc.scalar.dma_start(out=pk[:p, :], in_=inp[c0 : c0 + p, :])

    dist = pk[:p, 0:16]
    xs = pk[:p, 16:32]
    ys = pk[:p, 32:48]
    zs = pk[:p, 48:64]
    embi = pk[:p, 64:96]
    embj = pk[:p, 96:608]

    # 1 / dist  (dist in [0.5, 4.5], no zero risk)
    invd = tmp_pool.tile([P, 16], F32, tag="invd")
    nc.vector.reciprocal(invd[:p, :], dist)

    # emb_j[j,f] / dist[j]
    embjs = tmp_pool.tile([P, 512], F32, tag="embjs")
    nc.vector.tensor_tensor(
        out=embjs[:p, :].rearrange("p (j f) -> p j f", j=N),
        in0=embj.rearrange("p (j f) -> p j f", j=N),
        in1=invd[:p, :].unsqueeze(2).broadcast_to([p, N, F]),
        op=TT.mult,
    )

    # pairwise squared distance among the 16 neighbors
    a = tmp_pool.tile([P, 256], F32, tag="ta")
    b = tmp_pool.tile([P, 256], F32, tag="tb")
    dsq = tmp_pool.tile([P, 256], F32, tag="dsq")
    av = a[:p, :].rearrange("p (j k) -> p j k", j=N)
    bv = b[:p, :].rearrange("p (j k) -> p j k", j=N)
    dsqv = dsq[:p, :].rearrange("p (j k) -> p j k", j=N)
    for i, w in enumerate((xs, ys, zs)):
        wj = w.unsqueeze(2).broadcast_to([p, N, N])
        wk = w.unsqueeze(1).broadcast_to([p, N, N])
        nc.vector.tensor_tensor(out=av, in0=wj, in1=wk, op=TT.subtract)
        if i == 0:
            nc.vector.tensor_tensor(out=dsqv, in0=av, in1=av, op=TT.mult)
        else:
            nc.vector.tensor_tensor(out=bv, in0=av, in1=av, op=TT.mult)
            nc.vector.tensor_tensor(out=dsqv, in0=dsqv, in1=bv, op=TT.add)

    djk = tmp_pool.tile([P, 256], F32, tag="djk")
    nc.scalar.sqrt(djk[:p, :], dsq[:p, :])

    dij_b = dist.unsqueeze(2).broadcast_to([p, N, N])
    dik_b = dist.unsqueeze(1).broadcast_to([p, N, N])
    maxd = tmp_pool.tile([P, 256], F32, tag="maxd")
    mind = tmp_pool.tile([P, 256], F32, tag="mind")
    nc.vector.tensor_tensor(
        out=maxd[:p, :].rearrange("p (j k) -> p j k", j=N),
        in0=dij_b, in1=dik_b, op=TT.max,
    )
    nc.vector.tensor_tensor(
        out=mind[:p, :].rearrange("p (j k) -> p j k", j=N),
        in0=dij_b, in1=dik_b, op=TT.min,
    )
    rmin = tmp_pool.tile([P, 256], F32, tag="rmin")
    nc.vector.reciprocal(rmin[:p, :], mind[:p, :])
    # (djk - maxd) * rmin * 0.5 + 0.5  ==  (djk - maxd + mind)/(2*mind)
    nc.vector.tensor_tensor(out=b[:p, :], in0=djk[:p, :], in1=maxd[:p, :], op=TT.subtract)
    nc.vector.tensor_tensor(out=a[:p, :], in0=b[:p, :], in1=rmin[:p, :], op=TT.mult)
    tjk = tmp_pool.tile([P, 256], F32, tag="tjk")
    nc.scalar.activation(tjk[:p, :], a[:p, :], ACTF.Copy, bias=0.5, scale=0.5)

    embjs_v = embjs[:p, :].rearrange("p (j f) -> p j f", j=N)

    # ---- chunk A: flat rows [0, 136) = (j 0..7, k all) + (j=8, k 0..7)
    # 137 rows: one pad row so the 8x(17->16) off-diag view stays in bounds
    angA = ang_pool.tile([P, 137 * NF], F32, tag="ang")
    a1 = angA[:p, 0 : 128 * NF].rearrange("p (j k c) -> p j k c", j=8, c=NF)
    a2 = angA[:p, 128 * NF : 136 * NF].rearrange("p (k c) -> p k c", c=NF)
    aflat = angA[:p, 0 : 136 * NF].rearrange("p (r c) -> p r c", c=NF)

    # col 0: tij = dist[j], col 1: tik = dist[k], col 2: tjk
    nc.vector.tensor_copy(
        out=a1[:, :, :, 0], in_=dist[:, 0:8].unsqueeze(2).broadcast_to([p, 8, N])
    )
    nc.vector.tensor_copy(out=a2[:, :, 0], in_=dist[:, 8:9].broadcast_to([p, 8]))
    nc.vector.tensor_copy(
        out=a1[:, :, :, 1], in_=dist.unsqueeze(1).broadcast_to([p, 8, N])
    )
    nc.vector.tensor_copy(out=a2[:, :, 1], in_=dist[:, 0:8])
    nc.vector.tensor_copy(
        out=a1[:, :, :, 2], in_=tjk[:p, 0:128].rearrange("p (j k) -> p j k", j=8)
    )
    nc.vector.tensor_copy(out=a2[:, :, 2], in_=tjk[:p, 128:136])
    nc.scalar.copy(
        out=aflat[:, :, 3 : 3 + F], in_=embi.unsqueeze(1).broadcast_to([p, 136, F])
    )
    nc.vector.tensor_copy(
        out=a1[:, :, :, 3 + F : 3 + 2 * F],
        in_=embjs_v[:, 0:8, :].unsqueeze(2).broadcast_to([p, 8, N, F]),
    )
    nc.vector.tensor_copy(
        out=a2[:, :, 3 + F : 3 + 2 * F],
        in_=embjs_v[:, 8:9, :].broadcast_to([p, 8, F]),
    )
    nc.scalar.copy(
        out=a1[:, :, :, 3 + 2 * F : 3 + 3 * F],
        in_=embjs_v.unsqueeze(1).broadcast_to([p, 8, N, F]),
    )
    nc.scalar.copy(out=a2[:, :, 3 + 2 * F : 3 + 3 * F], in_=embjs_v[:, 0:8, :])
    # off-diag rows 1..135 skipping every 17th -> output rows 0..127
    nc.sync.dma_start(
        out=out[c0 : c0 + p, 0 : 128 * NF].rearrange("c (g x) -> c g x", g=8),
        in_=angA[:p, NF : NF + 8 * 17 * NF].rearrange("p (g x) -> p g x", g=8)[
            :, :, 0 : 16 * NF
        ],
    )

    # ---- chunk B: flat rows [136, 256) = (j=8, k 8..15) + (j 9..15, k all)
    angB = ang_pool.tile([P, 120 * NF], F32, tag="ang")
    b1 = angB[:p, 0 : 8 * NF].rearrange("p (k c) -> p k c", c=NF)
    b2 = angB[:p, 8 * NF : 120 * NF].rearrange("p (j k c) -> p j k c", j=7, c=NF)
    bflat = angB[:p, :].rearrange("p (r c) -> p r c", c=NF)  # r=120

    nc.vector.tensor_copy(out=b1[:, :, 0], in_=dist[:, 8:9].broadcast_to([p, 8]))
    nc.vector.tensor_copy(
        out=b2[:, :, :, 0], in_=dist[:, 9:16].unsqueeze(2).broadcast_to([p, 7, N])
    )
    nc.vector.tensor_copy(out=b1[:, :, 1], in_=dist[:, 8:16])
    nc.vector.tensor_copy(
        out=b2[:, :, :, 1], in_=dist.unsqueeze(1).broadcast_to([p, 7, N])
    )
    nc.vector.tensor_copy(out=b1[:, :, 2], in_=tjk[:p, 136:144])
    nc.vector.tensor_copy(
        out=b2[:, :, :, 2], in_=tjk[:p, 144:256].rearrange("p (j k) -> p j k", j=7)
    )
    nc.scalar.copy(
        out=bflat[:, :, 3 : 3 + F], in_=embi.unsqueeze(1).broadcast_to([p, 120, F])
    )
    nc.vector.tensor_copy(
        out=b1[:, :, 3 + F : 3 + 2 * F],
        in_=embjs_v[:, 8:9, :].broadcast_to([p, 8, F]),
    )
    nc.vector.tensor_copy(
        out=b2[:, :, :, 3 + F : 3 + 2 * F],
        in_=embjs_v[:, 9:16, :].unsqueeze(2).broadcast_to([p, 7, N, F]),
    )
    nc.scalar.copy(out=b1[:, :, 3 + 2 * F : 3 + 3 * F], in_=embjs_v[:, 8:16, :])
    nc.scalar.copy(
        out=b2[:, :, :, 3 + 2 * F : 3 + 3 * F],
        in_=embjs_v.unsqueeze(1).broadcast_to([p, 7, N, F]),
    )
    # off-diag rows 1..119 skipping every 17th -> output rows 128..239
    nc.gpsimd.dma_start(
        out=out[c0 : c0 + p, 128 * NF : 240 * NF].rearrange("c (g x) -> c g x", g=7),
        in_=angB[:p, NF : NF + 7 * 17 * NF].rearrange("p (g x) -> p g x", g=7)[
            :, :, 0 : 16 * NF
        ],
    )


def _build_nc():
    nc = bacc.Bacc("TRN2", target_bir_lowering=False, debug=False)
    inp = nc.declare_dram_parameter("packed", [C_CORE, ROW_IN], F32, isOutput=False)
    out = nc.declare_dram_parameter("out", [C_CORE, ROW_OUT], F32, isOutput=True)

    with tile.TileContext(nc) as tc:
        with (
            tc.tile_pool(name="io", bufs=3) as io_pool,
            tc.tile_pool(name="tmp", bufs=2) as tmp_pool,
            tc.tile_pool(name="ang", bufs=3) as ang_pool,
        ):
            c0 = 0
            while c0 < C_CORE:
                p = min(P, C_CORE - c0)
                _emit_tile(nc, io_pool, tmp_pool, ang_pool, inp, out, c0, p)
                c0 += p
    nc.compile()
    return nc


def _get_nc():
    global _CACHED_NC
    if _CACHED_NC is None:
        _CACHED_NC = _build_nc()
    return _CACHED_NC


def _pack_inputs(atoms_xyz, embed_table, dist_ij, atom_types, atom_i_idx, atom_j_idx):
    atoms_xyz = np.asarray(atoms_xyz, dtype=np.float32)
    embed_table = np.asarray(embed_table, dtype=np.float32)
    dist_ij = np.asarray(dist_ij, dtype=np.float32)
    atom_types = np.asarray(atom_types)
    atom_i_idx = np.asarray(atom_i_idx)
    atom_j_idx = np.asarray(atom_j_idx)

    C = dist_ij.shape[0]
    packed = np.empty((C, ROW_IN), dtype=np.float32)
    emb = embed_table[atom_types]                     # [nAtoms, F]
    xyz_j = atoms_xyz[atom_j_idx]                     # [C, N, 3]
    packed[:, 0:16] = dist_ij
    packed[:, 16:64] = np.ascontiguousarray(
        xyz_j.transpose(0, 2, 1)
    ).reshape(C, 48)                                  # x(16) y(16) z(16)
    packed[:, 64:96] = emb[atom_i_idx]                # emb_i
    packed[:, 96:608] = emb[atom_j_idx].reshape(C, N * F)
    return packed


def kernel(atoms_xyz, embed_table, dist_ij, atom_types, atom_i_idx, atom_j_idx):
    packed = _pack_inputs(
        atoms_xyz, embed_table, dist_ij, atom_types, atom_i_idx, atom_j_idx
    )
    nc = _get_nc()
    in_maps = [
        {"packed": packed[i * C_CORE : (i + 1) * C_CORE]} for i in range(NCORES)
    ]
    res = run_bass_kernel_spmd(nc, in_maps, list(range(NCORES)))
    outs = [res.results[i]["out"] for i in range(NCORES)]
    full = np.concatenate(outs, axis=0).reshape(C_TOT, NROW, NF)
    centers = np.asarray(atom_i_idx).reshape(-1)
    return full, centers
